# revision 16
# baseline (speedup 1.0000x reference)
"""HRM (two-level GRU) Trainium2 kernel.

Strategy: data-parallel over batch across the 8 NeuronCores (B=8 -> one batch
element per core). The GRU recurrence is inherently serial over the 256 steps;
each core runs its own element's recurrence independently (zero inter-core
communication). Logits (the bulk of the FLOPs) are computed per-core for that
core's batch element with out_W streamed from HBM in bf16.

Layouts (per core, batch element b):
  - All K=1024 contractions pack dim d -> (partition d%128, k-tile d//128).
  - Recurrence matmul: streaming form, out[1, 3072] = lhsT(h^T [128,1]).T @
    rhs(W^T [128, n]), accumulated over 8 k-tiles; 4 column-groups of the PE
    array run concurrently via tile_position (4x throughput at M=1). Loop is
    k-major so the four groups' streams interleave.
  - Gate math happens partition-major on [128, 24] tiles (gate dim g ->
    (g%128, g//128)) after a PSUM->SBUF copy + one reshape DMA.
"""

import os
import sys
import numpy as np

for _p in ("/opt/trn_rl_repo", "/root/.axon_site/_ro/trn_rl_repo"):
    if _p not in sys.path and os.path.isdir(_p):
        sys.path.append(_p)

import ml_dtypes  # noqa: E402

BF16 = np.float16

B, S, V, H, T = 8, 256, 32000, 1024, 4
G3 = 3 * H            # 3072 gate dims
KT = H // 128         # 8 k-tiles
GC = G3 // 128        # 24 gate chunks
N_CORES = 8
NBLK = int(os.environ.get("KERNEL_NBLK", S // T))  # 64 blocks (dev override)
NTOK = NBLK * T

NG = 4                # PE column groups for the M=1 streaming matmuls
NPER = G3 // NG       # 768 N-columns per group
CHUNKS = [(0, 512), (512, 256)]  # PSUM bank is 512 fp32

LOG_NCH = 512         # logits vocab chunk

_cache = {}


def _build():
    import concourse.bass as bass  # noqa: F401
    import concourse.mybir as mybir
    import concourse.tile as tile
    from concourse import bacc
    from contextlib import ExitStack

    f32 = mybir.dt.float32
    bf16 = mybir.dt.float16

    nc = bacc.Bacc(num_devices=N_CORES)

    # ---- parameters (per-core shards prepared on host) ----
    xT = nc.declare_dram_parameter("xT", [H, NTOK], bf16, isOutput=False)
    WxT = nc.declare_dram_parameter("WxT", [H, G3], bf16, isOutput=False)
    WhT = nc.declare_dram_parameter("WhT", [H, G3], bf16, isOutput=False)
    WhhT = nc.declare_dram_parameter("WhhT", [H, G3], bf16, isOutput=False)
    hWihT = nc.declare_dram_parameter("hWihT", [H, G3], bf16, isOutput=False)
    hWhhT = nc.declare_dram_parameter("hWhhT", [H, G3], bf16, isOutput=False)
    outWT = nc.declare_dram_parameter("outWT", [H, V], bf16, isOutput=False)
    bias24 = nc.declare_dram_parameter("bias24", [128, GC], f32, isOutput=False)
    biasH24 = nc.declare_dram_parameter("biasH24", [128, GC], f32, isOutput=False)
    bhhN = nc.declare_dram_parameter("bhhN", [128, KT], f32, isOutput=False)
    hbhhN = nc.declare_dram_parameter("hbhhN", [128, KT], f32, isOutput=False)
    outB = nc.declare_dram_parameter("outB", [1, V], bf16, isOutput=False)
    hl0 = nc.declare_dram_parameter("hl0", [128, KT], f32, isOutput=False)
    hh0 = nc.declare_dram_parameter("hh0", [128, KT], f32, isOutput=False)

    logits_out = nc.declare_dram_parameter("logits_out", [NTOK, V], f32, isOutput=True)
    hl_out = nc.declare_dram_parameter("hl_out", [128, KT], f32, isOutput=True)
    hh_out = nc.declare_dram_parameter("hh_out", [128, KT], f32, isOutput=True)

    # internal DRAM: precomputed x-part of gi, laid out [chunk, token, partition]
    giX_d = nc.dram_tensor("giX_d", [GC, NTOK, 128], f32)

    with tile.TileContext(nc) as tc, ExitStack() as top:
        res = top.enter_context(tc.tile_pool(name="resident", bufs=1))

        whh_sb = res.tile([128, KT, G3], bf16)     # 6.3 MB
        hwih_sb = res.tile([128, KT, G3], bf16)    # 6.3 MB
        hwhh_sb = res.tile([128, KT, G3], bf16)    # 6.3 MB
        nc.sync.dma_start(whh_sb[:], WhhT.ap().rearrange("(k p) n -> p k n", p=128))
        nc.sync.dma_start(hwih_sb[:], hWihT.ap().rearrange("(k p) n -> p k n", p=128))
        nc.sync.dma_start(hwhh_sb[:], hWhhT.ap().rearrange("(k p) n -> p k n", p=128))

        hsT = res.tile([128, KT, NTOK], bf16)      # h_l history (matmul lhsT)
        state = res.tile([128, KT], f32)           # h_l (fp32)
        state_h = res.tile([128, KT], f32)         # h_h (fp32)
        hinit_bf = res.tile([128, KT], bf16)
        hh_bf = res.tile([128, KT], bf16)
        b24 = res.tile([128, GC], f32)
        bh24 = res.tile([128, GC], f32)
        bhhn_sb = res.tile([128, KT], f32)
        hbhhn_sb = res.tile([128, KT], f32)

        nc.sync.dma_start(state[:], hl0[:, :])
        nc.sync.dma_start(state_h[:], hh0[:, :])
        nc.sync.dma_start(b24[:], bias24[:, :])
        nc.sync.dma_start(bh24[:], biasH24[:, :])
        nc.sync.dma_start(bhhn_sb[:], bhhN[:, :])
        nc.sync.dma_start(hbhhn_sb[:], hbhhN[:, :])
        nc.vector.tensor_copy(out=hinit_bf[:], in_=state[:])
        nc.vector.tensor_copy(out=hh_bf[:], in_=state_h[:])

        # ---------- phase 1: giX = (Wx @ x^T + bias), stored to DRAM ----------
        with ExitStack() as ph1:
            p1 = ph1.enter_context(tc.tile_pool(name="pre", bufs=3))
            p1ps = ph1.enter_context(tc.tile_pool(name="preps", bufs=2, space="PSUM"))
            xt_sb = p1.tile([128, KT, NTOK], bf16, tag="xt", bufs=1)
            nc.sync.dma_start(xt_sb[:], xT.ap().rearrange("(k p) t -> p k t", p=128))
            for mt in range(GC):
                lw = p1.tile([128, KT, 128], bf16, tag="wx")
                nc.sync.dma_start(
                    lw[:],
                    WxT.ap()[:, mt * 128:(mt + 1) * 128].rearrange(
                        "(k p) m -> p k m", p=128
                    ),
                )
                ps = p1ps.tile([128, NTOK], f32, tag="ps")
                for k in range(KT):
                    nc.tensor.matmul(
                        ps[:, :],
                        lw[:, k, :],
                        xt_sb[:, k, :],
                        start=(k == 0),
                        stop=(k == KT - 1),
                    )
                stg1 = p1.tile([128, NTOK], f32, tag="stg1")
                nc.vector.tensor_add(
                    stg1[:, :],
                    ps[:, :],
                    b24[:, mt: mt + 1].to_broadcast((128, NTOK)),
                )
                nc.sync.dma_start(
                    giX_d[mt].rearrange("t p -> p t"), stg1[:, :]
                )

        # ---------- phase 2: the recurrence ----------
        with ExitStack() as ph2:
            wk = ph2.enter_context(tc.tile_pool(name="whT", bufs=1))
            stg = ph2.enter_context(tc.tile_pool(name="stage", bufs=3))
            gat = ph2.enter_context(tc.tile_pool(name="gates", bufs=2))
            ps_s = ph2.enter_context(tc.tile_pool(name="ps_s", bufs=2, space="PSUM"))
            ps_h = ph2.enter_context(tc.tile_pool(name="ps_h", bufs=1, space="PSUM"))
            drp = ph2.enter_context(tc.tile_pool(name="drsh", bufs=2, space="DRAM"))

            def mm_3072(psum, lhsT_col, rhs_sb):
                """psum[32g, :768] (4 col-groups) = lhsT.T @ rhs over k-tiles.

                k-major issue order so the 4 column-group streams overlap.
                """
                for k in range(KT):
                    for g in range(NG):
                        for off, cw in CHUNKS:
                            nc.tensor.matmul(
                                psum[32 * g: 32 * g + 1, off: off + cw],
                                lhsT_col(k),
                                rhs_sb(k, g * NPER + off, cw),
                                start=(k == 0),
                                stop=(k == KT - 1),
                                tile_position=(0, 32 * g),
                            )

            def psum_to_24(psum, tag):
                """PSUM [rows 32g, 768] -> SBUF gate-major [128, 24]."""
                st = stg.tile([128, NPER], f32, tag="st")
                nc.vector.tensor_copy(out=st[0:1, :], in_=psum[0:1, :])
                nc.scalar.copy(st[32:33, :], psum[32:33, :])
                nc.vector.tensor_copy(out=st[64:65, :], in_=psum[64:65, :])
                nc.scalar.copy(st[96:97, :], psum[96:97, :])
                g24 = gat.tile([128, GC], f32, tag=f"g24_{tag}")
                rsh = drp.tile([NG, NPER], f32, tag="rsh")
                nc.sync.dma_start(rsh[:, :], st[0:128:32, :])
                nc.sync.dma_start(
                    g24[:].rearrange("p (g c) -> p g c", g=NG),
                    rsh[:, :].rearrange("g (c p) -> p g c", p=128),
                )
                return g24

            def gru_gates(gh24, gib, bhh_n, st_f32, cast_out):
                """st = (1-z)*n + z*st; cast_out <- bf16(st)."""
                t1 = gat.tile([128, KT], f32, tag="t1")
                t2 = gat.tile([128, KT], f32, tag="t2")
                r = gat.tile([128, KT], f32, tag="r")
                z = gat.tile([128, KT], f32, tag="z")
                nc.vector.tensor_add(t1[:], gh24[:, 0:KT], gib[:, 0:KT])
                nc.scalar.activation(r[:], t1[:], mybir.ActivationFunctionType.Sigmoid)
                nc.vector.tensor_add(t2[:], gh24[:, KT:2 * KT], gib[:, KT:2 * KT])
                nc.scalar.activation(z[:], t2[:], mybir.ActivationFunctionType.Sigmoid)
                hn = gat.tile([128, KT], f32, tag="hn")
                nc.vector.tensor_add(hn[:], gh24[:, 2 * KT:3 * KT], bhh_n)
                nc.vector.tensor_mul(hn[:], r[:], hn[:])
                nc.vector.tensor_add(hn[:], hn[:], gib[:, 2 * KT:3 * KT])
                n = gat.tile([128, KT], f32, tag="n")
                nc.scalar.activation(n[:], hn[:], mybir.ActivationFunctionType.Tanh)
                d = gat.tile([128, KT], f32, tag="d")
                nc.vector.tensor_sub(d[:], st_f32[:], n[:])
                nc.vector.tensor_mul(d[:], z[:], d[:])
                nc.vector.tensor_add(st_f32[:], n[:], d[:])
                nc.vector.tensor_copy(out=cast_out, in_=st_f32[:])

            for blk in range(NBLK):
                # -- stream WhT for this block's gihh --
                wh_tiles = []
                for g in range(NG):
                    wt = wk.tile([128, KT, NPER], bf16, tag=f"wh{g}")
                    nc.sync.dma_start(
                        wt[:],
                        WhT.ap()[:, g * NPER:(g + 1) * NPER].rearrange(
                            "(k p) n -> p k n", p=128
                        ),
                    )
                    wh_tiles.append(wt)

                # -- gihh = Wh @ h_h^T --
                ps_g = ps_s.tile([128, NPER], f32, tag="ps_step")
                mm_3072(
                    ps_g,
                    lambda k: hh_bf[:, k: k + 1],
                    lambda k, n0, cw: wh_tiles[n0 // NPER][
                        :, k, n0 % NPER: n0 % NPER + cw
                    ],
                )
                gihh24 = psum_to_24(ps_g, "gihh")

                # -- T inner steps --
                for t in range(T):
                    tok = blk * T + t
                    prev = hinit_bf if tok == 0 else None
                    ps_t = ps_s.tile([128, NPER], f32, tag="ps_step")

                    def lh(k, _tok=tok, _prev=prev):
                        if _prev is not None:
                            return _prev[:, k: k + 1]
                        return hsT[:, k, _tok - 1: _tok]

                    gix_t = gat.tile([128, GC], f32, tag="gix_t")
                    nc.sync.dma_start(
                        gix_t[:, :],
                        giX_d.ap()[:, tok, :].rearrange("c p -> p c"),
                    )
                    gib = gat.tile([128, GC], f32, tag="gib")
                    nc.vector.tensor_add(gib[:, :], gix_t[:, :], gihh24[:, :])
                    mm_3072(ps_t, lh, lambda k, n0, cw: whh_sb[:, k, n0: n0 + cw])
                    gh24 = psum_to_24(ps_t, "step")
                    gru_gates(gh24, gib, bhhn_sb[:], state, hsT[:, :, tok])

                # -- H-GRU: h_h = GRU(x=h_l, h=h_h) --
                t3 = blk * T + T - 1
                ps_a = ps_h.tile([128, NPER], f32, tag="ps_a")
                ps_b = ps_h.tile([128, NPER], f32, tag="ps_b")
                mm_3072(
                    ps_a,
                    lambda k: hsT[:, k, t3: t3 + 1],
                    lambda k, n0, cw: hwih_sb[:, k, n0: n0 + cw],
                )
                mm_3072(
                    ps_b,
                    lambda k: hh_bf[:, k: k + 1],
                    lambda k, n0, cw: hwhh_sb[:, k, n0: n0 + cw],
                )
                a24 = psum_to_24(ps_a, "ha")
                b24h = psum_to_24(ps_b, "hb")
                nc.vector.tensor_add(a24[:, :], a24[:, :], bh24[:, :])
                ghr = gat.tile([128, KT], f32, tag="ghr")
                ghz = gat.tile([128, KT], f32, tag="ghz")
                nc.vector.tensor_add(ghr[:], a24[:, 0:KT], b24h[:, 0:KT])
                nc.vector.tensor_add(ghz[:], a24[:, KT:2 * KT], b24h[:, KT:2 * KT])
                rH = gat.tile([128, KT], f32, tag="rH")
                zH = gat.tile([128, KT], f32, tag="zH")
                nc.scalar.activation(
                    rH[:], ghr[:], mybir.ActivationFunctionType.Sigmoid
                )
                nc.scalar.activation(
                    zH[:], ghz[:], mybir.ActivationFunctionType.Sigmoid
                )
                hnH = gat.tile([128, KT], f32, tag="hnH")
                nc.vector.tensor_add(hnH[:], b24h[:, 2 * KT:3 * KT], hbhhn_sb[:])
                nc.vector.tensor_mul(hnH[:], rH[:], hnH[:])
                nc.vector.tensor_add(hnH[:], hnH[:], a24[:, 2 * KT:3 * KT])
                nH = gat.tile([128, KT], f32, tag="nH")
                nc.scalar.activation(nH[:], hnH[:], mybir.ActivationFunctionType.Tanh)
                dH = gat.tile([128, KT], f32, tag="dH")
                nc.vector.tensor_sub(dH[:], state_h[:], nH[:])
                nc.vector.tensor_mul(dH[:], zH[:], dH[:])
                nc.vector.tensor_add(state_h[:], nH[:], dH[:])
                nc.vector.tensor_copy(out=hh_bf[:], in_=state_h[:])

            nc.sync.dma_start(hl_out[:, :], state[:])
            nc.sync.dma_start(hh_out[:, :], state_h[:])

        # ---------- phase 3: logits = hs @ out_W^T + out_b ----------
        with ExitStack() as ph3:
            lg = ph3.enter_context(tc.tile_pool(name="logits", bufs=3))
            lps = ph3.enter_context(tc.tile_pool(name="lps", bufs=4, space="PSUM"))
            ones_sb = lg.tile([1, 128], bf16, tag="ones", bufs=1)
            nc.vector.memset(ones_sb[:], 1.0)
            MT = max(1, NTOK // 128)
            MROWS = min(128, NTOK)
            nchunks = (V + LOG_NCH - 1) // LOG_NCH
            for c in range(nchunks):
                c0 = c * LOG_NCH
                cw = min(LOG_NCH, V - c0)
                wch = lg.tile([128, KT, LOG_NCH], bf16, tag="wch")
                nc.sync.dma_start(
                    wch[:, :, :cw],
                    outWT.ap()[:, c0: c0 + cw].rearrange("(k p) n -> p k n", p=128),
                )
                obt = lg.tile([1, LOG_NCH], bf16, tag="obt")
                nc.sync.dma_start(obt[:, :cw], outB.ap()[:, c0: c0 + cw])
                for m in range(MT):
                    ps = lps.tile([128, LOG_NCH], f32, tag="lgps")
                    # bias via K=1 ones matmul: ps[p, n] = out_b[n]
                    nc.tensor.matmul(
                        ps[:MROWS, :cw],
                        ones_sb[0:1, :MROWS],
                        obt[0:1, :cw],
                        start=True,
                        stop=False,
                    )
                    for k in range(KT):
                        nc.tensor.matmul(
                            ps[:MROWS, :cw],
                            hsT[:, k, m * 128: m * 128 + MROWS],
                            wch[:, k, :cw],
                            start=False,
                            stop=(k == KT - 1),
                        )
                    ls = lg.tile([128, LOG_NCH], f32, tag="lstage")
                    nc.vector.tensor_copy(out=ls[:MROWS, :cw], in_=ps[:MROWS, :cw])
                    nc.sync.dma_start(
                        logits_out[m * 128: m * 128 + MROWS, c0: c0 + cw],
                        ls[:MROWS, :cw],
                    )

    nc.compile()
    return nc


def _get_nc():
    if "nc" not in _cache:
        _cache["nc"] = _build()
    return _cache["nc"]


def _prep_inputs(x, h_l_init, h_h_init, emb, l_Wih, l_Whh, l_bih, l_bhh,
                 h_Wih, h_Whh, h_bih, h_bhh, out_W, out_b):
    """Host-side sharding: one batch element per core."""
    x = np.asarray(x)
    emb = np.asarray(emb, dtype=np.float32)
    x_emb = emb[x]                       # [B, S, H]
    l_Wih = np.asarray(l_Wih, np.float32)
    l_Whh = np.asarray(l_Whh, np.float32)
    h_Wih = np.asarray(h_Wih, np.float32)
    h_Whh = np.asarray(h_Whh, np.float32)
    l_bih = np.asarray(l_bih, np.float32)
    l_bhh = np.asarray(l_bhh, np.float32)
    h_bih = np.asarray(h_bih, np.float32)
    h_bhh = np.asarray(h_bhh, np.float32)
    out_W = np.asarray(out_W, np.float32)
    out_b = np.asarray(out_b, np.float32)

    def kmaj(v):  # [1024] -> [128, KT]
        return np.ascontiguousarray(v.reshape(KT, 128).T)

    def gmaj(v):  # [3072] -> [128, GC]
        return np.ascontiguousarray(v.reshape(GC, 128).T)

    bias = l_bih.copy()
    bias[:2 * H] += l_bhh[:2 * H]        # bhh for r/z folds into gi bias
    biasH = h_bih.copy()
    biasH[:2 * H] += h_bhh[:2 * H]

    shared = {
        "WxT": np.ascontiguousarray(l_Wih[:, :H].T).astype(BF16),
        "WhT": np.ascontiguousarray(l_Wih[:, H:].T).astype(BF16),
        "WhhT": np.ascontiguousarray(l_Whh.T).astype(BF16),
        "hWihT": np.ascontiguousarray(h_Wih.T).astype(BF16),
        "hWhhT": np.ascontiguousarray(h_Whh.T).astype(BF16),
        "outWT": np.ascontiguousarray(out_W.T).astype(BF16),
        "bias24": gmaj(bias).astype(np.float32),
        "biasH24": gmaj(biasH).astype(np.float32),
        "bhhN": kmaj(l_bhh[2 * H:]).astype(np.float32),
        "hbhhN": kmaj(h_bhh[2 * H:]).astype(np.float32),
        "outB": out_b.reshape(1, V).astype(BF16),
    }
    in_maps = []
    for b in range(N_CORES):
        m = dict(shared)
        m["xT"] = np.ascontiguousarray(x_emb[b, :NTOK].T).astype(BF16)
        m["hl0"] = kmaj(np.asarray(h_l_init, np.float32)[b]).astype(np.float32)
        m["hh0"] = kmaj(np.asarray(h_h_init, np.float32)[b]).astype(np.float32)
        in_maps.append(m)
    return in_maps


def kernel(**inputs):
    from concourse.bass_utils import run_bass_kernel_spmd

    nc = _get_nc()
    in_maps = _prep_inputs(**inputs)
    res = run_bass_kernel_spmd(nc, in_maps, core_ids=list(range(N_CORES)))

    outputs = np.stack(
        [np.asarray(res.results[b]["logits_out"], np.float32) for b in range(N_CORES)]
    )  # [B, NTOK, V]
    h_l = np.stack(
        [np.asarray(res.results[b]["hl_out"]).T.reshape(H) for b in range(N_CORES)]
    )
    h_h = np.stack(
        [np.asarray(res.results[b]["hh_out"]).T.reshape(H) for b in range(N_CORES)]
    )
    return outputs, h_l, h_h
